# revision 1
# baseline (speedup 1.0000x reference)
"""Trainium2 Bass kernel for gated GQA attention (Qwen3.5-style block).

Full inputs -> full outputs. Internally shards batch over 4 cores (fsdp) x
heads over 2 cores (tp): core c handles batch c//2, head-half c%2
(16 q-heads / 4 kv-heads). Host sums the two tp partial outputs per batch.

v2: single fused pipeline, fp8 DoubleRow projections.
  - All projections (q/gate/k/v/o) run as 3-pass hi/lo-compensated fp8e4
    DoubleRow matmuls (256-deep contraction per step, 0.5 cyc/row):
    W@x ~= Wh@xh + Wh@xl + Wl@xh, with x and W quantized on the host
    (scales SX/SW powers of two, folded out in the PSUM drains).
  - No DRAM scratch: k/v/q/gate/attn all stay in SBUF; per-head attention
    is interleaved with the q projections so PE never waits on a phase.
  - Softmax denominators and rms-norm sums use gpsimd partition_all_reduce
    (replicated output kills the ones-broadcast matmuls); the RoPE
    partition rotate-by-40 uses a DVE stream_shuffle instead of a PE
    permutation matmul.
  - Attention output is quantized hi/lo to fp8 on device and the o-proj
    runs DoubleRow over head pairs.
"""
import sys, os
sys.path.insert(0, '/opt/trn_rl_repo')
from contextlib import ExitStack

import numpy as np
import ml_dtypes

import concourse.bass as bass
import concourse.tile as tile
from concourse import bacc, mybir, bass_isa
from concourse.bass_utils import run_bass_kernel_spmd

F32R = mybir.dt.float32r
F32 = mybir.dt.float32
F16 = mybir.dt.float16
F8 = mybir.dt.float8e4
E4 = ml_dtypes.float8_e4m3
AF = mybir.ActivationFunctionType
DR = mybir.MatmulPerfMode.DoubleRow

B, T, D, N, K, H = 4, 1024, 4096, 32, 8, 128
ROT = 80          # rotary dims per head
RH = ROT // 2     # 40
THETA = 1.0e6
EPS = 1e-6
NL = N // 2       # 16 q heads per core
KL = K // 2       # 4 kv heads per core
NP = D // 256     # 16 DoubleRow contraction pairs
TQ = 256          # token tile for attention columns
NCOL = T // TQ    # 4
NTT = T // 128    # 8 token tiles of 128
ND = D // 512     # 8 output d tiles
HP = NL // 2      # 8 head pairs for o-proj

SX = 16.0         # x fp8 scale
SW = 1024.0       # q/k/v/o weight fp8 scale
SA = 16.0         # attention-out fp8 scale (folded into v drain)
SXW = SX * SW

N_CORES = 8

# stream_shuffle mask: groups of 4 partitions; rotate by 40 (10 groups):
# out group i reads in group mask[i]. rows 0..39 <- 40..79, 40..79 <- 0..39.
SHUF40 = [i + 10 for i in range(10)] + [i for i in range(10)] + list(range(20, 32))

_NC_CACHE = {}


def build_nc():
    PH = int(os.environ.get("KM_PHASES", "3"))
    key = ("nc", PH)
    if key in _NC_CACHE:
        return _NC_CACHE[key]
    nc = bacc.Bacc("TRN2", target_bir_lowering=False, debug=False)

    # ---- DRAM I/O ----
    xh_d = nc.dram_tensor("xh", [NP, 128, 2, T], F8, kind="ExternalInput").ap()
    xl_d = nc.dram_tensor("xl", [NP, 128, 2, T], F8, kind="ExternalInput").ap()
    wqh_d = nc.dram_tensor("wqh", [NL, 2, 128, NP, 2, 128], F8, kind="ExternalInput").ap()
    wql_d = nc.dram_tensor("wql", [NL, 2, 128, NP, 2, 128], F8, kind="ExternalInput").ap()
    wkh_d = nc.dram_tensor("wkh", [KL, 128, NP, 2, 128], F8, kind="ExternalInput").ap()
    wkl_d = nc.dram_tensor("wkl", [KL, 128, NP, 2, 128], F8, kind="ExternalInput").ap()
    wvh_d = nc.dram_tensor("wvh", [NP, 128, 2, KL * 128], F8, kind="ExternalInput").ap()
    wvl_d = nc.dram_tensor("wvl", [NP, 128, 2, KL * 128], F8, kind="ExternalInput").ap()
    woh_d = nc.dram_tensor("woh", [ND, 128, HP, 2, 512], F8, kind="ExternalInput").ap()
    wol_d = nc.dram_tensor("wol", [ND, 128, HP, 2, 512], F8, kind="ExternalInput").ap()
    cosq_d = nc.dram_tensor("cosq", [128, T], F16, kind="ExternalInput").ap()
    sinq_d = nc.dram_tensor("sinq", [ROT, T], F16, kind="ExternalInput").ap()
    cosk_d = nc.dram_tensor("cosk", [128, T], F16, kind="ExternalInput").ap()
    sink_d = nc.dram_tensor("sink", [ROT, T], F16, kind="ExternalInput").ap()
    masks_d = nc.dram_tensor("masks", [2, 128, TQ], F16, kind="ExternalInput").ap()
    epsq_d = nc.dram_tensor("epsq", [128, 1], F32, kind="ExternalInput").ap()
    epsk_d = nc.dram_tensor("epsk", [128, 1], F32, kind="ExternalInput").ap()
    out_d = nc.dram_tensor("out", [T, D], F32, kind="ExternalOutput").ap()

    with tile.TileContext(nc) as tc:
      with ExitStack() as ctx:
        cpool = ctx.enter_context(tc.tile_pool(name="consts", bufs=1))
        c_m0 = cpool.tile([128, TQ], F16)
        nc.sync.dma_start(c_m0, masks_d[0])
        c_m1 = cpool.tile([128, TQ], F16)
        nc.sync.dma_start(c_m1, masks_d[1])
        c_epsq = cpool.tile([128, 1], F32)
        nc.sync.dma_start(c_epsq, epsq_d)
        c_epsk = cpool.tile([128, 1], F32)
        nc.sync.dma_start(c_epsk, epsk_d)

        # attn out fp8 pairs live through the o-proj phase
        a8pool = ctx.enter_context(tc.tile_pool(name="a8", bufs=HP))
        a8h, a8l = [], []
        for hp in range(HP):
            th = a8pool.tile([128, NTT, 2, 128], F8, tag="a8h", name=f"a8h{hp}")
            a8h.append(th)
            tl = a8pool.tile([128, NTT, 2, 128], F8, tag="a8l", name=f"a8l{hp}")
            a8l.append(tl)

        with ExitStack() as p1:
            tabpool = p1.enter_context(tc.tile_pool(name="tables", bufs=1))
            c_cosq = tabpool.tile([128, T], F16)
            nc.sync.dma_start(c_cosq, cosq_d)
            c_sinq = tabpool.tile([ROT, T], F16)
            nc.sync.dma_start(c_sinq, sinq_d)
            c_cosk = tabpool.tile([128, T], F16)
            nc.sync.dma_start(c_cosk, cosk_d)
            c_sink = tabpool.tile([ROT, T], F16)
            nc.sync.dma_start(c_sink, sink_d)

            xpool = p1.enter_context(tc.tile_pool(name="xt", bufs=NP))
            xh, xl = [], []
            for p in range(NP):
                t = xpool.tile([128, 2, T], F8, tag="xh", name=f"xh{p}")
                nc.sync.dma_start(t, xh_d[p])
                xh.append(t)
            for p in range(NP):
                t = xpool.tile([128, 2, T], F8, tag="xl", name=f"xl{p}")
                nc.sync.dma_start(t, xl_d[p])
                xl.append(t)

            kvpool = p1.enter_context(tc.tile_pool(name="kv", bufs=max(KL, NTT)))
            wspool = p1.enter_context(tc.tile_pool(name="wslab", bufs=2))
            qpool = p1.enter_context(tc.tile_pool(name="qg", bufs=2))
            wpool = p1.enter_context(tc.tile_pool(name="work", bufs=2))
            epool = p1.enter_context(tc.tile_pool(name="ew", bufs=2))
            psq = p1.enter_context(tc.tile_pool(name="psq", bufs=4, space="PSUM"))
            psa = p1.enter_context(tc.tile_pool(name="psa", bufs=2, space="PSUM"))

            def project_tile(whi, wlo):
                """96 DoubleRow matmuls into two [128,512] psums (token
                halves), consecutive pairs sharing the stationary slab so the
                PE weight load amortizes over 1024 moving rows."""
                psA = psq.tile([128, 512], F32, tag="pq", bufs=4, name="psprA")
                psB = psq.tile([128, 512], F32, tag="pq", bufs=4, name="psprB")
                steps = ([(whi, xh[p], p) for p in range(NP)] +
                         [(whi, xl[p], p) for p in range(NP)] +
                         [(wlo, xh[p], p) for p in range(NP)])
                last = len(steps) - 1
                for i, (w, x, p) in enumerate(steps):
                    nc.tensor.matmul(psA, w[:, p], x[:, :, 0:512],
                                     start=(i == 0), stop=(i == last),
                                     perf_mode=DR)
                    nc.tensor.matmul(psB, w[:, p], x[:, :, 512:1024],
                                     start=(i == 0), stop=(i == last),
                                     perf_mode=DR)
                return psA, psB

            def drain_norm(psA, psB, c_cos, c_sin, c_eps, kscale, dst):
                """RMS-norm + partial RoPE a [128,T] projected head into dst."""
                for half, ps in ((0, psA), (1, psB)):
                    sl = slice(half * 512, half * 512 + 512)
                    sbq = wpool.tile([128, 512], F32R, tag="sbq", bufs=2)
                    nc.scalar.activation(sbq, ps, AF.Copy, scale=float(1.0 / SXW))
                    q2h = wpool.tile([128, 512], F32, tag="q2h", bufs=1)
                    nc.vector.tensor_mul(q2h, sbq, sbq)
                    ssr = wpool.tile([128, 512], F32, tag="ssr", bufs=2)
                    nc.gpsimd.partition_all_reduce(ssr, q2h, channels=128,
                                                   reduce_op=bass_isa.ReduceOp.add)
                    sqv = wpool.tile([128, 512], F32, tag="sqv", bufs=1)
                    if kscale is None:
                        nc.scalar.activation(sqv, ssr, AF.Sqrt, bias=c_eps)
                    else:
                        nc.scalar.activation(sqv, ssr, AF.Sqrt, bias=c_eps,
                                             scale=float(kscale))
                    rrw = wpool.tile([128, 512], F32R, tag="rrw", bufs=2)
                    with nc.allow_low_precision(reason="f32r output is f32-width"):
                        nc.vector.reciprocal(rrw, sqv)
                    swp = wpool.tile([128, 512], F32R, tag="swp", bufs=1)
                    nc.sync.dma_start(swp[0:RH], sbq[RH:ROT])
                    nc.sync.dma_start(swp[RH:ROT], sbq[0:RH])
                    qfh = wpool.tile([128, 512], F32R, tag="qfh", bufs=2)
                    nc.vector.tensor_mul(qfh, sbq, c_cos[:, sl])
                    t2h = wpool.tile([ROT, 512], F32, tag="t2h", bufs=1)
                    nc.vector.tensor_mul(t2h, swp[0:ROT], c_sin[:, sl])
                    nc.vector.tensor_add(qfh[0:ROT], qfh[0:ROT], t2h)
                    nc.vector.tensor_mul(dst[:, sl], qfh, rrw)

            # ---------- K projection ----------
            kf = []
            for kv in range(KL):
                whk = wspool.tile([128, NP, 2, 128], F8, tag="wh", bufs=2)
                nc.sync.dma_start(whk, wkh_d[kv])
                wlk = wspool.tile([128, NP, 2, 128], F8, tag="wl", bufs=2)
                nc.sync.dma_start(wlk, wkl_d[kv])
                psA, psB = project_tile(whk, wlk)
                kf_t = kvpool.tile([128, T], F16, tag="kf", bufs=KL, name=f"kf{kv}")
                drain_norm(psA, psB, c_cosk, c_sink, c_epsk, 1.0 / H, kf_t)
                kf.append(kf_t)

            # ---------- V projection (tokens on psum partitions) ----------
            # 2 groups of 4 token-tiles; wv pair slabs streamed in a nested
            # scope (pair-major so each slab is consumed by adjacent steps)
            vall = [kvpool.tile([128, KL * 128], F16, tag="vt", bufs=NTT,
                                name=f"v{tt}") for tt in range(NTT)]
            with ExitStack() as pv_scope:
                wvpool = pv_scope.enter_context(
                    tc.tile_pool(name="wvslab", bufs=3))
                for ttg in range(2):
                    tts = [ttg * 4 + i for i in range(4)]
                    pvs = [psq.tile([128, 512], F32, tag="pq", bufs=4,
                                    name=f"psv{tt}") for tt in tts]
                    for p in range(NP):
                        wvh_t = wvpool.tile([128, 2, KL * 128], F8, tag="wvh",
                                            bufs=3)
                        nc.sync.dma_start(wvh_t, wvh_d[p])
                        wvl_t = wvpool.tile([128, 2, KL * 128], F8, tag="wvl",
                                            bufs=3)
                        nc.sync.dma_start(wvl_t, wvl_d[p])
                        # xh-stationary serves both wvh and wvl back-to-back
                        for i, tt in enumerate(tts):
                            tsl = slice(tt * 128, tt * 128 + 128)
                            nc.tensor.matmul(
                                pvs[i], xh[p][:, :, tsl], wvh_t,
                                start=(p == 0), stop=False, perf_mode=DR)
                            nc.tensor.matmul(
                                pvs[i], xh[p][:, :, tsl], wvl_t,
                                start=False, stop=False, perf_mode=DR)
                        for i, tt in enumerate(tts):
                            tsl = slice(tt * 128, tt * 128 + 128)
                            nc.tensor.matmul(
                                pvs[i], xl[p][:, :, tsl], wvh_t,
                                start=False, stop=(p == NP - 1),
                                perf_mode=DR)
                    for i, tt in enumerate(tts):
                        # fold the attn-out fp8 scale SA into v
                        nc.scalar.activation(vall[tt], pvs[i], AF.Copy,
                                             scale=float(SA / SXW))

            # ---------- Q heads: project + attention, interleaved ----------
            for n in range(NL):
                kv = n // 4
                whq = wspool.tile([128, NP, 2, 128], F8, tag="wh", bufs=2)
                nc.sync.dma_start(whq, wqh_d[n, 0])
                wlq = wspool.tile([128, NP, 2, 128], F8, tag="wl", bufs=2)
                nc.sync.dma_start(wlq, wql_d[n, 0])
                psA, psB = project_tile(whq, wlq)
                qf_t = qpool.tile([128, T], F16, tag="qf", bufs=2)
                drain_norm(psA, psB, c_cosq, c_sinq, c_epsq, None, qf_t)

                whg = wspool.tile([128, NP, 2, 128], F8, tag="wh", bufs=2)
                nc.sync.dma_start(whg, wqh_d[n, 1])
                wlg = wspool.tile([128, NP, 2, 128], F8, tag="wl", bufs=2)
                nc.sync.dma_start(wlg, wql_d[n, 1])
                psGA, psGB = project_tile(whg, wlg)
                gate_t = qpool.tile([128, T], F16, tag="gat", bufs=2)
                nc.scalar.activation(gate_t[:, 0:512], psGA, AF.Sigmoid,
                                     scale=float(1.0 / SXW))
                nc.scalar.activation(gate_t[:, 512:1024], psGB, AF.Sigmoid,
                                     scale=float(1.0 / SXW))

                hp, slot = n // 2, n % 2
                for j in range(NCOL if PH >= 2 else 0):
                    ns = 2 * j + 2
                    jsl = slice(j * TQ, (j + 1) * TQ)
                    ppv = psa.tile([128, TQ], F32, tag="pv", bufs=2, name="ppv")
                    eacc = epool.tile([128, TQ], F32, tag="eacc", bufs=2)
                    qcol = qf_t[:, jsl]
                    for si in range(ns):
                        ps_sc = psa.tile([128, TQ], F32, tag="sc", bufs=2,
                                         name="psc")
                        nc.tensor.matmul(ps_sc,
                                         kf[kv][:, si * 128:(si + 1) * 128],
                                         qcol, start=True, stop=True)
                        e_t = epool.tile([128, TQ], F16, tag="et", bufs=8)
                        if si >= ns - 2:
                            er = epool.tile([128, TQ], F32, tag="er", bufs=2)
                            nc.scalar.activation(er, ps_sc, AF.Exp)
                            mt = c_m0 if si == ns - 2 else c_m1
                            nc.vector.tensor_mul(e_t, er, mt)
                        else:
                            nc.scalar.activation(e_t, ps_sc, AF.Exp)
                        nc.tensor.matmul(ppv,
                                         vall[si][:, kv * 128:(kv + 1) * 128],
                                         e_t, start=(si == 0), stop=(si == ns - 1))
                        if si == 0:
                            nc.vector.tensor_copy(eacc, e_t)
                        else:
                            nc.vector.tensor_add(eacc, eacc, e_t)
                    dnr = epool.tile([128, TQ], F32, tag="dnr", bufs=2)
                    nc.gpsimd.partition_all_reduce(dnr, eacc, channels=128,
                                                   reduce_op=bass_isa.ReduceOp.add)
                    rcp = epool.tile([128, TQ], F32R, tag="rcp", bufs=2)
                    with nc.allow_low_precision(reason="f32r output is f32-width"):
                        nc.vector.reciprocal(rcp, dnr)
                    tmp = epool.tile([128, TQ], F32, tag="tmp", bufs=2)
                    nc.vector.tensor_mul(tmp, ppv, gate_t[:, jsl])
                    atsa = epool.tile([128, TQ], F32, tag="atsa", bufs=2)
                    nc.vector.tensor_mul(atsa, tmp, rcp)
                    # hi/lo fp8 quantize into the o-proj pair layout
                    # (contiguous per token-tile so the o-proj stationary
                    # slices are contiguous in SBUF)
                    deq = epool.tile([128, TQ], F32, tag="deq", bufs=1)
                    alo = epool.tile([128, TQ], F32, tag="alo", bufs=1)
                    for t2 in range(2):
                        tq = 2 * j + t2
                        csl = slice(t2 * 128, t2 * 128 + 128)
                        nc.scalar.activation(a8h[hp][:, tq, slot, :],
                                             atsa[:, csl], AF.Copy)
                        nc.scalar.activation(deq[:, csl],
                                             a8h[hp][:, tq, slot, :], AF.Copy)
                    nc.vector.tensor_sub(alo, atsa, deq)
                    for t2 in range(2):
                        tq = 2 * j + t2
                        csl = slice(t2 * 128, t2 * 128 + 128)
                        nc.scalar.activation(a8l[hp][:, tq, slot, :],
                                             alo[:, csl], AF.Copy)

        # ---------- O projection ----------
        # d-tiles processed in pairs so each a8 stationary load serves two
        # 512-row matmuls (one per d-tile)
        with ExitStack() as p3:
          if PH >= 3:
            wopool = p3.enter_context(tc.tile_pool(name="wo", bufs=4))
            opool = p3.enter_context(tc.tile_pool(name="osb", bufs=6))
            pso = p3.enter_context(tc.tile_pool(name="pso", bufs=6, space="PSUM"))
            for dp in range(ND // 2):
                d0, d1 = 2 * dp, 2 * dp + 1
                wo_h0 = wopool.tile([128, HP, 2, 512], F8, tag="woh0", bufs=2)
                nc.sync.dma_start(wo_h0, woh_d[d0])
                wo_l0 = wopool.tile([128, HP, 2, 512], F8, tag="wol0", bufs=2)
                nc.sync.dma_start(wo_l0, wol_d[d0])
                wo_h1 = wopool.tile([128, HP, 2, 512], F8, tag="woh1", bufs=2)
                nc.sync.dma_start(wo_h1, woh_d[d1])
                wo_l1 = wopool.tile([128, HP, 2, 512], F8, tag="wol1", bufs=2)
                nc.sync.dma_start(wo_l1, wol_d[d1])
                for tq in range(NTT):
                    ps0 = pso.tile([128, 512], F32, tag="po", bufs=6,
                                   name=f"po{d0}_{tq}")
                    ps1 = pso.tile([128, 512], F32, tag="po", bufs=6,
                                   name=f"po{d1}_{tq}")
                    steps = ([(a8h, wo_h0, wo_h1, hp) for hp in range(HP)] +
                             [(a8l, wo_h0, wo_h1, hp) for hp in range(HP)] +
                             [(a8h, wo_l0, wo_l1, hp) for hp in range(HP)])
                    last = len(steps) - 1
                    tql = slice(tq * 128, (tq + 1) * 128)
                    for i, (a8, w0, w1, hp) in enumerate(steps):
                        nc.tensor.matmul(ps0, a8[hp][:, tq], w0[:, hp],
                                         start=(i == 0), stop=(i == last),
                                         perf_mode=DR)
                        nc.tensor.matmul(ps1, a8[hp][:, tq], w1[:, hp],
                                         start=(i == 0), stop=(i == last),
                                         perf_mode=DR)
                    for d, ps in ((d0, ps0), (d1, ps1)):
                        osb = opool.tile([128, 512], F32, tag="osb", bufs=6)
                        nc.scalar.activation(osb, ps, AF.Copy,
                                             scale=float(1.0 / (SA * SW)))
                        nc.sync.dma_start(out_d[tql, d * 512:(d + 1) * 512], osb)

    nc.compile()
    _NC_CACHE[key] = nc
    return nc


def _rope_tables(pos, norm_w):
    """cos [128,T] and sin [80,T] f16 tables with (1+w) folded in."""
    pos = pos.astype(np.float32)
    fraction = (2.0 * np.arange(RH, dtype=np.float32) / np.float32(ROT))
    timescale = np.power(np.float32(THETA), fraction).astype(np.float32)
    ang = (pos[None, :] / timescale[:, None]).astype(np.float32)  # [40, T]
    cosv = np.cos(ang).astype(np.float32)
    sinv = np.sin(ang).astype(np.float32)
    w1 = 1.0 + norm_w.astype(np.float32)  # [128]
    cos_t = np.ones((128, pos.shape[0]), np.float32)
    cos_t[0:RH] = cosv
    cos_t[RH:ROT] = cosv
    cos_t *= w1[:, None]
    sin_t = np.empty((ROT, pos.shape[0]), np.float32)
    sin_t[0:RH] = -sinv * w1[RH:ROT, None]   # partner is h+40
    sin_t[RH:ROT] = sinv * w1[0:RH, None]    # partner is h-40
    return cos_t.astype(np.float16), sin_t.astype(np.float16)


def _q8(a, scale):
    """hi/lo e4m3 split of a*scale; returns (hi8, lo8)."""
    s = (np.asarray(a, np.float32) * np.float32(scale))
    hi8 = s.astype(E4)
    lo8 = (s - hi8.astype(np.float32)).astype(E4)
    return hi8, lo8


def _pack_w_stationary(w, sw):
    """[D, M] -> hi/lo [128, NP, 2, M] fp8 stationary slabs."""
    d, m = w.shape
    hi8, lo8 = _q8(w, sw)

    def pk(a8):
        return np.ascontiguousarray(
            a8.reshape(NP, 2, 128, m).transpose(2, 0, 1, 3))
    return pk(hi8), pk(lo8)


def make_in_maps(x, positions, wq, wk, wv, wo, q_norm_w, k_norm_w):
    f_idx = np.arange(TQ)[None, :]
    p_idx = np.arange(128)[:, None]
    masks = np.stack([(p_idx <= f_idx).astype(np.float16),
                      (p_idx + 128 <= f_idx).astype(np.float16)])

    # per head-half weight maps (shared across the 4 batches)
    half_maps = []
    for half in range(2):
        n0, k0 = half * NL, half * KL
        wqh = np.empty((NL, 2, 128, NP, 2, 128), E4)
        wql = np.empty((NL, 2, 128, NP, 2, 128), E4)
        for n in range(NL):
            for g in range(2):
                W = wq[:, n0 + n, g * 128:(g + 1) * 128]
                wqh[n, g], wql[n, g] = _pack_w_stationary(W, SW)
        wkh = np.empty((KL, 128, NP, 2, 128), E4)
        wkl = np.empty((KL, 128, NP, 2, 128), E4)
        for kv in range(KL):
            wkh[kv], wkl[kv] = _pack_w_stationary(wk[:, k0 + kv, :], SW)
        # v: moving side [NP, 128, 2, 512]
        Wv = wv[:, k0:k0 + KL, :].reshape(D, KL * 128)
        vh8, vl8 = _q8(Wv, SW)

        def pkv(a8):
            return np.ascontiguousarray(
                a8.reshape(NP, 2, 128, KL * 128).transpose(0, 2, 1, 3))
        wvh, wvl = pkv(vh8), pkv(vl8)
        # o: moving side [ND, 128, HP, 2, 512]
        Wo = wo[n0:n0 + NL]                       # [NL, H, D]
        oh8, ol8 = _q8(Wo, SW)

        def pko(a8):
            # [NL, 128, D] -> [ND, 128(H), HP, 2(slot), 512]
            a = a8.reshape(HP, 2, 128, ND, 512)
            return np.ascontiguousarray(a.transpose(3, 2, 0, 1, 4))
        woh, wol = pko(oh8), pko(ol8)
        half_maps.append({
            "wqh": wqh, "wql": wql, "wkh": wkh, "wkl": wkl,
            "wvh": wvh, "wvl": wvl, "woh": woh, "wol": wol,
        })

    in_maps = []
    for c in range(N_CORES):
        b, half = c // 2, c % 2
        xt = np.ascontiguousarray(x[b].T)          # [D, T]
        xh8, xl8 = _q8(xt, SX)

        def pkx(a8):
            return np.ascontiguousarray(
                a8.reshape(NP, 2, 128, T).transpose(0, 2, 1, 3))
        cq, sq = _rope_tables(positions[b], q_norm_w)
        ck, sk = _rope_tables(positions[b], k_norm_w)
        m = {
            "xh": pkx(xh8), "xl": pkx(xl8),
            "cosq": cq, "sinq": sq, "cosk": ck, "sink": sk,
            "masks": masks,
            "epsq": np.full((128, 1), H * EPS, np.float32),
            "epsk": np.full((128, 1), EPS, np.float32),
        }
        m.update(half_maps[half])
        in_maps.append(m)
    return in_maps


def _wait_devices_healthy(attempts=8, sleep_s=15):
    """The axon-tunneled devices occasionally report NRT_EXEC_UNIT_UNRECOVERABLE
    transiently (e.g. after an aborted process); they recover on retry."""
    import jax, time
    for attempt in range(attempts):
        try:
            jax.block_until_ready(
                [jax.device_put(np.ones(4, np.float32), d) + 1
                 for d in jax.devices()[:N_CORES]])
            return
        except Exception:
            if attempt == attempts - 1:
                raise
            time.sleep(sleep_s)


def kernel(x, positions, wq, wk, wv, wo, q_norm_w, k_norm_w):
    import time
    x = np.asarray(x, np.float32)
    positions = np.asarray(positions)
    wq = np.asarray(wq, np.float32)
    wk = np.asarray(wk, np.float32)
    wv = np.asarray(wv, np.float32)
    wo = np.asarray(wo, np.float32)
    q_norm_w = np.asarray(q_norm_w, np.float32)
    k_norm_w = np.asarray(k_norm_w, np.float32)

    nc = build_nc()
    in_maps = make_in_maps(x, positions, wq, wk, wv, wo, q_norm_w, k_norm_w)
    _wait_devices_healthy()
    res = None
    for attempt in range(3):
        try:
            res = run_bass_kernel_spmd(nc, in_maps, core_ids=list(range(N_CORES)))
            break
        except Exception:
            if attempt == 2:
                raise
            time.sleep(20)
            _wait_devices_healthy()
    out = np.empty((B, T, D), np.float32)
    for b in range(B):
        out[b] = res.results[2 * b]["out"] + res.results[2 * b + 1]["out"]
    return out

